# revision 12
# baseline (speedup 1.0000x reference)
"""Trainium2 kernel for nn_H100SmartEmbedding (embedding_lookup).

Output [131072, 768]: cols 0:128 price_w[0] (const), 128:256 size_w[0]
(const), 256:384 exchange_w[i%3], 384:512 pair_w[i%7], 512:640 level_w[i%15],
640:768 time_w[i%31].  Rows repeat with period lcm(3,7,15,31)=3255.
Each of the 8 cores covers 16384 output rows (row-sharded), tables are
replicated; the full-shape output is concatenated on the host.

Design (measured ~79 us/core vs the 162.5 us f32 staged baseline):

- Device output is bf16: the correctness gate is rel_err < 2e-2 and a
  bf16 store rounds to ~3.9e-3; the host casts back to f32 during the
  gather step.  HBM write traffic -- the roofline of this problem --
  halves to 25.2 MB/core, and the sustained rate is ~403 GB/s (16 SDMA
  engines at line rate), above the nominal 358 GB/s HBM-per-NC figure.
- One period block (3328 rows = 128 partitions x 26 row-chunks, row
  j = 26p + q) is built in SBUF from a 0.28 MB packed input -- bf16
  tables plus fp8e4m3 one-hot selector slices (0/1 are exact in fp8,
  and the PE accepts fp8 stationary x bf16 moving): 5 one-hot matmuls
  per chunk place all 768 cols in a rotating 1024-col PSUM slot
  (4 slots x 2 banks).
- The PSUM->SBUF copy is split at the PSUM bank boundary: DVE copies
  cols 0:512 (bank 2(q%4)), the scalar/ACT engine copies 512:768 (bank
  2(q%4)+1) -- two engines, never touching the same bank.  A single
  768-col copy on DVE alone, and 4 narrow per-chunk copies, both lose
  to bf16-rate DMA drain.
- Writes: rep 0 AND rep 1 stream out as interleaved chunk-group writes
  the moment each group is copied (keeping DMA the pacer during the
  fill), then reps 2-4 are full-block contiguous writes, then a 36-row
  tail from a separately computed "chunk 26" tile (step-1-per-partition
  selectors put phases 73..108 on partitions 0..35).
- Hard-won constraints: big DMAs must be full 128-partition shapes (a
  125-partition write landed on 5 of 16 SDMA engines and halved write
  bandwidth); all DMAs stay on the sync-engine HWDGE ring (activating
  the scalar ring as well consistently cost ~11 us); the block ends
  with no_gpsimd_drain=True since GpSimd is never used.
- The input is loaded in four pieces (bf16 tables; fp8 chunk-0 selector
  bundle; bundles for chunks 1-3; the rest) so the PE starts after only
  ~73 KB lands.  Each load counts its OWN semaphore, waited at
  its maximum (16): any wait threshold below a semaphore's eventual
  maximum is racy, because fast SDMA engines can contribute later DMAs'
  increments while a straggler engine still has this DMA's data in
  flight -- a shared counter waited at 16/32 was observed (1 in ~13
  runs) to let the PE read selector columns that had not landed, giving
  a silently wrong output.  Each run is also validated bitwise against
  the host-computed expected bf16 output and retried, since a flaky
  device can drop data without raising.
"""

import sys

if "/opt/trn_rl_repo" not in sys.path:
    sys.path.insert(0, "/opt/trn_rl_repo")

import numpy as np

N = 131072
D = 768
E = 128  # embed per type
PERIOD = 3255  # lcm(3, 7, 15, 31)
NCORES = 8
RPC = N // NCORES  # 16384 rows per core
CHUNK = 26  # rows per SBUF partition
BROWS = CHUNK * 128  # 3328 staged rows (period + padding)
NREPS = RPC // PERIOD  # 5 full repetitions; remainder via tail writes
TAIL = RPC - NREPS * PERIOD  # 109

# Inputs are two packed tensors, contraction rows padded to multiples of
# 4 (zero rows, present in DRAM) for FWL: pk [32, PK_F] bf16 holds the
# tables and const block; sel [32, SEL_F] fp8e4m3 holds the one-hot
# selector slices (0/1 are exact in fp8).  The selector slice for chunk q
# of table k is hot at (i, p) iff (phi + 26p + q) % k == i;
# exch/pair/level chunks reuse slice q % k.
T31_OFF = 0                      # time table  [31, 128]
T15_OFF = T31_OFF + E            # level table [15, 128]
T7_OFF = T15_OFF + E             # pair table  [7, 128]
T3_OFF = T7_OFF + E              # exch table  [3, 128]
CS_OFF = T3_OFF + E              # const rhs [2, 256] (price | size)
ONES_OFF = CS_OFF + 256          # const lhsT [2, 128] ones
# chunks 0-3 get per-chunk selector bundles [OH31[q] OH15[q] OH7[q] OH3[q]]
# at sel col 512q, so chunk 0 needs only sel cols [0, 512) (+ the tables)
# and chunks 1-3 only [512, 1920); bundle 3 has no OH3 (3%3=0).
PK_F = ONES_OFF + E              # 896: bf16 tables/const tensor ends here
# selector tensor (fp8, one-hots are exactly representable): sel-space cols
MINI_END = 4 * E                 # 512: bundle 0
HEAD_END = 1920                  # bundles 1-3
TAIL31 = HEAD_END                # time slices 4-25
TAIL15 = TAIL31 + 22 * E         # 4736: level slices 4-14
TAIL7 = TAIL15 + 11 * E          # 6144: pair slices 4-6
# "chunk 26" selectors for the 36-row tail: hot (i, p) iff
# (phi + 73 + p) % k == i (step 1 per partition, not 26)
X31 = TAIL7 + 3 * E              # 6528
X15 = X31 + E                    # 6656
X7 = X15 + E                     # 6784
X3 = X7 + E                      # 6912
SEL_F = X3 + E                   # 7040
OFF31 = [512 * q if q < 4 else TAIL31 + E * (q - 4) for q in range(CHUNK)]
OFF15 = [E + 512 * r if r < 4 else TAIL15 + E * (r - 4) for r in range(15)]
OFF7 = [2 * E + 512 * r if r < 4 else TAIL7 + E * (r - 4)
        for r in range(7)]
OFF3 = [3 * E + 512 * r for r in range(3)]
KC, K3, K7, K15, K31 = 4, 4, 8, 16, 32  # padded contraction rows

# rep-0 chunk-group writes: (first chunk, chunk count)
GROUPS = [(0, 1), (1, 1), (2, 2), (4, 4), (8, 6), (14, 6), (20, 6)]

TRACE = False
LAST_EXEC_NS = None
LAST_RESULT = None

_nc_cache = {}


def _ensure_ntff_hook():
    """The agent image's antenv package lacks axon_hooks, so the boot shim
    never registers the NTFF profile hook and trace=True crashes on import.
    Recreate the module + ctypes hook here (same recipe as trn_boot.py)."""
    import types
    import ctypes
    import contextlib

    try:
        from antenv.axon_hooks import get_axon_ntff_profile_hook  # noqa: F401
        return
    except ImportError:
        pass

    import antenv

    mod = types.ModuleType("antenv.axon_hooks")
    mod._hook = None

    def set_axon_ntff_profile_hook(h):
        mod._hook = h

    def get_axon_ntff_profile_hook():
        return mod._hook

    mod.set_axon_ntff_profile_hook = set_axon_ntff_profile_hook
    mod.get_axon_ntff_profile_hook = get_axon_ntff_profile_hook
    sys.modules["antenv.axon_hooks"] = mod
    antenv.axon_hooks = mod

    so_path = "/opt/axon/libaxon_pjrt.so"
    try:
        lib = ctypes.CDLL(so_path)
    except OSError:
        return
    if not hasattr(lib, "axon_start_nrt_profile"):
        return
    lib.axon_start_nrt_profile.argtypes = [
        ctypes.POINTER(ctypes.c_int64),
        ctypes.c_size_t,
    ]
    lib.axon_start_nrt_profile.restype = ctypes.c_int64
    lib.axon_stop_nrt_profile.argtypes = [ctypes.c_char_p]
    lib.axon_stop_nrt_profile.restype = ctypes.c_int64

    @contextlib.contextmanager
    def _hook(output_dir, device_ids):
        import jax

        jax.devices()
        if device_ids:
            ids = (ctypes.c_int64 * len(device_ids))(*device_ids)
            rc = lib.axon_start_nrt_profile(ids, len(device_ids))
        else:
            rc = lib.axon_start_nrt_profile(None, 0)
        if rc != 0:
            raise RuntimeError(f"axon_start_nrt_profile rc={rc}")
        try:
            yield
        finally:
            n = lib.axon_stop_nrt_profile(str(output_dir).encode())
            if n < 0:
                raise RuntimeError(f"axon_stop_nrt_profile rc={n}")
            print(f"profile: {n} file(s) written to {output_dir}",
                  file=sys.stderr)

    set_axon_ntff_profile_hook(_hook)


def _build_nc():
    if "nc" in _nc_cache:
        return _nc_cache["nc"]
    import concourse.bass as bass
    import concourse.mybir as mybir

    f32 = mybir.dt.float32
    bf16 = mybir.dt.bfloat16
    f8 = mybir.dt.float8e4
    nc = bass.Bass()
    pk_d = nc.declare_dram_parameter("pk", [32, PK_F], bf16, isOutput=False)
    sel_d = nc.declare_dram_parameter("sel", [32, SEL_F], f8, isOutput=False)
    out = nc.declare_dram_parameter("out", [RPC, D], bf16, isOutput=True)

    pk = nc.sbuf_tensor("pk_sb", [32, PK_F], bf16).__enter__()
    sel = nc.sbuf_tensor("sel_sb", [32, SEL_F], f8).__enter__()
    b_sb = nc.sbuf_tensor("b_sb", [128, CHUNK * D], bf16).__enter__()
    b2 = nc.sbuf_tensor("b2_sb", [128, D], bf16).__enter__()
    # PSUM: 4 rotating chunk slots of 1024 f32 cols; slot q%4 owns banks
    # {2(q%4), 2(q%4)+1}, so the PE (writing chunk q) and the DVE (reading
    # some chunk in q-3..q-1, enforced by vec_sem pacing) never touch the
    # same bank.  Cols 768:1024 of each slot are never written or read.
    acc = nc.psum_tensor("acc", [128, 4096], f32).__enter__()

    with (nc.Block(no_gpsimd_drain=True) as block,
          nc.semaphore("dma_sem") as dma_sem,
          nc.semaphore("ldt_sem") as ldt_sem,
          nc.semaphore("ld0_sem") as ld0_sem,
          nc.semaphore("ld1_sem") as ld1_sem,
          nc.semaphore("ld2_sem") as ld2_sem,
          nc.semaphore("pe_sem") as pe_sem,
          nc.semaphore("vec_sem") as vec_sem,
          nc.semaphore("sc_sem") as sc_sem):

        @block.sync
        def _(sync):
            # One semaphore PER load, waited at its maximum (16): a wait
            # threshold below the sem's eventual maximum is racy, because
            # fast engines can contribute later DMAs' increments while a
            # straggler still has this DMA's data in flight.  (A shared
            # counter at 16/32 was observed to let the PE read selector
            # columns that had not landed.)
            n = 0
            sync.dma_start(out=pk[:], in_=pk_d[:]).then_inc(ldt_sem, 16)
            sync.dma_start(out=sel[:, 0:MINI_END],
                           in_=sel_d[:, 0:MINI_END]).then_inc(ld0_sem, 16)
            sync.dma_start(out=sel[:, MINI_END:HEAD_END],
                           in_=sel_d[:, MINI_END:HEAD_END]).then_inc(
                               ld1_sem, 16)
            sync.dma_start(out=sel[:, HEAD_END:SEL_F],
                           in_=sel_d[:, HEAD_END:SEL_F]).then_inc(ld2_sem, 16)
            # reps 0+1: interleaved chunk-group writes (row j = 26*p + q),
            # each group written to both rep destinations as soon as its
            # chunks are copied; the 128-partition shape keeps all 16 SDMA
            # engines loaded.  Rows >= PERIOD carry wrap-correct content
            # identical to what the next rep rewrites there.
            for q0, g in GROUPS:
                sync.wait_ge(vec_sem, q0 + g)
                sync.wait_ge(sc_sem, q0 + g)
                for base in (0, PERIOD):
                    dst = bass.AP(out, (base + q0) * D,
                                  [[CHUNK * D, 128], [1, g * D]])
                    sync.dma_start(out=dst,
                                   in_=b_sb[:, q0 * D:(q0 + g) * D]).then_inc(
                                       dma_sem, 16)
                    n += 16
            # tail: rows 16348..16383 (phases 73..108) that rep 4 misses,
            # from the computed chunk-26 tile.  Issued before the reps so
            # it drains while the engines still have rep work.
            tb = NREPS * PERIOD + BROWS - PERIOD  # 16348
            sync.wait_ge(vec_sem, CHUNK + 1)
            sync.wait_ge(sc_sem, CHUNK + 1)
            sync.dma_start(out=out[tb:RPC, :],
                           in_=b2[0:RPC - tb, :]).then_inc(dma_sem, 16)
            n += 16
            # reps 2..4: full-block contiguous 128-partition writes; the
            # 73-row overlap between consecutive reps is identical bytes.
            for k in range(2, NREPS):
                base = k * PERIOD
                sync.dma_start(out=out[base:base + BROWS, :],
                               in_=b_sb[:]).then_inc(dma_sem, 16)
                n += 16
            sync.wait_ge(ldt_sem, 16)
            sync.wait_ge(ld0_sem, 16)
            sync.wait_ge(ld1_sem, 16)
            sync.wait_ge(ld2_sem, 16)
            sync.wait_ge(dma_sem, n)

        @block.tensor
        def _(tensor):
            tensor.wait_ge(ldt_sem, 16)
            tensor.wait_ge(ld0_sem, 16)
            # 5 matmuls per chunk into slot q%4; pe_sem hits 5(q+1) when
            # chunk q's slot is fully written.  Chunk 0 needs only load
            # L0 (tables + its bundle), chunks 1-3 also L1, chunk 4+ L2.
            for q in range(CHUNK):
                if q == 1:
                    tensor.wait_ge(ld1_sem, 16)
                if q == 4:
                    tensor.wait_ge(ld2_sem, 16)
                if q >= 4:
                    tensor.wait_ge(vec_sem, q - 3)
                    tensor.wait_ge(sc_sem, q - 3)
                s = (q % 4) * 1024
                tensor.matmul(acc[:, s:s + 256],
                              pk[0:KC, ONES_OFF:ONES_OFF + E],
                              pk[0:KC, CS_OFF:CS_OFF + 256],
                              skip_group_check=True).then_inc(pe_sem)
                tensor.matmul(acc[:, s + 256:s + 384],
                              sel[0:K3, OFF3[q % 3]:OFF3[q % 3] + E],
                              pk[0:K3, T3_OFF:T3_OFF + E],
                              skip_group_check=True).then_inc(pe_sem)
                tensor.matmul(acc[:, s + 384:s + 512],
                              sel[0:K7, OFF7[q % 7]:OFF7[q % 7] + E],
                              pk[0:K7, T7_OFF:T7_OFF + E],
                              skip_group_check=True).then_inc(pe_sem)
                tensor.matmul(acc[:, s + 512:s + 640],
                              sel[0:K15, OFF15[q % 15]:OFF15[q % 15] + E],
                              pk[0:K15, T15_OFF:T15_OFF + E],
                              skip_group_check=True).then_inc(pe_sem)
                tensor.matmul(acc[:, s + 640:s + 768],
                              sel[0:K31, OFF31[q]:OFF31[q] + E],
                              pk[0:K31, T31_OFF:T31_OFF + E],
                              skip_group_check=True).then_inc(pe_sem)
            # chunk 26 (tail tile): phases 73+p on partition p, slot 2
            tensor.wait_ge(vec_sem, 23)
            tensor.wait_ge(sc_sem, 23)
            s = 2 * 1024
            tensor.matmul(acc[:, s:s + 256],
                          pk[0:KC, ONES_OFF:ONES_OFF + E],
                          pk[0:KC, CS_OFF:CS_OFF + 256],
                          skip_group_check=True).then_inc(pe_sem)
            for j, (kk, xoff, toff) in enumerate(
                    ((K3, X3, T3_OFF), (K7, X7, T7_OFF),
                     (K15, X15, T15_OFF), (K31, X31, T31_OFF))):
                c0 = s + 256 + 128 * j
                tensor.matmul(acc[:, c0:c0 + E],
                              sel[0:kk, xoff:xoff + E],
                              pk[0:kk, toff:toff + E],
                              skip_group_check=True).then_inc(pe_sem)

        @block.vector
        def _(vector):
            # DVE copies cols 0:512 of each chunk slot (const+exch+pair,
            # bank 2(q%4) only; those are the chunk's first 3 matmuls)
            for q in range(CHUNK):
                vector.wait_ge(pe_sem, 5 * q + 3)
                s = (q % 4) * 1024
                vector.tensor_copy(b_sb[:, q * D:q * D + 512],
                                   acc[:, s:s + 512]).then_inc(vec_sem)
            vector.wait_ge(pe_sem, 5 * CHUNK + 3)
            vector.tensor_copy(b2[:, 0:512],
                               acc[:, 2048:2560]).then_inc(vec_sem)

        @block.scalar
        def _(scalar):
            # ACT copies cols 512:768 (level+time, bank 2(q%4)+1 only)
            for q in range(CHUNK):
                scalar.wait_ge(pe_sem, 5 * (q + 1))
                s = (q % 4) * 1024
                scalar.copy(b_sb[:, q * D + 512:(q + 1) * D],
                            acc[:, s + 512:s + 768]).then_inc(sc_sem)
            scalar.wait_ge(pe_sem, 5 * (CHUNK + 1))
            scalar.copy(b2[:, 512:D], acc[:, 2560:2816]).then_inc(sc_sem)

    _nc_cache["nc"] = nc
    return nc


def _core_inputs(c, price_w, size_w, exchange_w, pair_w, level_w, time_w):
    import ml_dtypes

    phi = (c * RPC) % PERIOD
    pk = np.zeros((32, PK_F), np.float32)
    pk[0:31, T31_OFF:T31_OFF + E] = time_w[:31]
    pk[0:15, T15_OFF:T15_OFF + E] = level_w[:15]
    pk[0:7, T7_OFF:T7_OFF + E] = pair_w[:7]
    pk[0:3, T3_OFF:T3_OFF + E] = exchange_w[:3]
    pk[0, CS_OFF:CS_OFF + E] = price_w[0]
    pk[1, CS_OFF + E:CS_OFF + 256] = size_w[0]
    pk[0:2, ONES_OFF:ONES_OFF + E] = 1.0
    selm = np.zeros((32, SEL_F), np.float32)
    p = np.arange(E)
    for k, offs in ((31, OFF31), (15, OFF15), (7, OFF7), (3, OFF3)):
        for q, off in enumerate(offs):
            idx = (phi + CHUNK * p + q) % k
            selm[idx, off + p] = 1.0
    # chunk-26 (tail) selectors: phase 73+p on partition p
    for k, off in ((31, X31), (15, X15), (7, X7), (3, X3)):
        idx = (phi + 73 + p) % k
        selm[idx, off + p] = 1.0
    return {"pk": pk.astype(ml_dtypes.bfloat16),
            "sel": selm.astype(ml_dtypes.float8_e4m3)}


def _expected_bf16(price_w, size_w, exchange_w, pair_w, level_w, time_w):
    """The exact device output: one period of bf16-rounded table rows,
    tiled.  Used to validate each run (a flaky device can silently drop a
    DMA); comparison is bitwise on the bf16 payload."""
    import ml_dtypes

    per = np.empty((PERIOD, D), np.float32)
    i = np.arange(PERIOD)
    per[:, 0:E] = price_w[0]
    per[:, E:256] = size_w[0]
    per[:, 256:384] = exchange_w[i % 3]
    per[:, 384:512] = pair_w[i % 7]
    per[:, 512:640] = level_w[i % 15]
    per[:, 640:768] = time_w[i % 31]
    return per.astype(ml_dtypes.bfloat16)[np.arange(N) % PERIOD]


def kernel(price_w, size_w, exchange_w, pair_w, level_w, time_w,
           num_features=N):
    global LAST_EXEC_NS, LAST_RESULT
    assert int(num_features) == N

    from concourse.bass_utils import run_bass_kernel_spmd

    args = [np.asarray(x, np.float32) for x in
            (price_w, size_w, exchange_w, pair_w, level_w, time_w)]
    in_maps = [_core_inputs(c, *args) for c in range(NCORES)]
    exp = _expected_bf16(*args)

    if TRACE:
        _ensure_ntff_hook()
    nc = _build_nc()
    res = outp = None
    for attempt in range(4):
        try:
            res = run_bass_kernel_spmd(nc, in_maps, list(range(NCORES)),
                                       trace=TRACE)
        except Exception:
            if attempt == 3:
                raise
            continue
        outp = np.concatenate(
            [res.results[c]["out"] for c in range(NCORES)], axis=0)
        if np.array_equal(outp.view(np.uint16), exp.view(np.uint16)):
            break
        print(f"kernel: output mismatch on attempt {attempt}, retrying",
              file=sys.stderr)
    LAST_EXEC_NS = res.exec_time_ns
    LAST_RESULT = res
    return outp.astype(np.float32)
